# revision 2
# baseline (speedup 1.0000x reference)
"""Trainium2 Bass kernel for a basic RNN:
    h_t = W_hh @ tanh(h_{t-1}) + W_ih @ x_t   (pre-activation hidden stored)
    x: [B=64, T=512, NIN=256] fp32, W_ih: [512, 256], W_hh: [512, 512]
    out: [B, T, N=512] fp32

Strategy
--------
Data-parallel over batch: B=64 -> 8 cores x BL=8 sequences each.
Per core everything is kept in a hidden-major ("transposed") layout
[hidden (partition), batch (free)] so the sequential recurrence needs no
per-step transposes:

  - host pre-transposes x to xT[c, t*BL+b] and the weights to W.T
  - PE computes xp.T = W_ih.T^T @ x.T for all steps (one big matmul)
  - recurrence: for each t, for each output chunk m (4x128):
        psum[m] = sum_k W_hh.T[k-chunk, m-chunk]^T @ tanh(h_{t-1}).T[k-chunk]
    with W_hh tiles as the stationary operand (bf16 -> fast weight load)
    and the tiny [128, BL] activation as the moving operand.
    DVE adds xp_t, ACT applies tanh (emitting bf16 for the next step).
  - h_t (fp32, hidden-major) is staged in SBUF and DMA'd out in blocks;
    the host transposes back to [B, T, N].

The W_hh stream through the PE array (128 elem/cycle) is the intrinsic
floor: ~853ns/step * 512 steps.
"""

import os
import numpy as np
import ml_dtypes

B, T, NIN, N = 64, 512, 256, 512
NCORES = 8
BL = B // NCORES  # 8 sequences per core
KC = N // 128  # 4 hidden chunks
CC = NIN // 128  # 2 input-feature chunks
TBLK = 64  # steps staged in SBUF between output DMAs

# "bf16" (fast) or "f32" (exact, ~4x slower recurrence) or "f32r"
REC_DTYPE = os.environ.get("RNN_REC_DTYPE", "bf16")
PROJ_DTYPE = os.environ.get("RNN_PROJ_DTYPE", "f32r")

_CACHE = {}


def _build():
    import concourse.bacc as bacc
    import concourse.mybir as mybir
    from concourse import tile

    dt = mybir.dt
    f32 = dt.float32
    bf16 = dt.bfloat16

    rec_w_dt = {"bf16": bf16, "f32": f32, "f32r": f32}[REC_DTYPE]
    rec_mm_dt = {"bf16": bf16, "f32": f32, "f32r": dt.float32r}[REC_DTYPE]
    proj_mm_dt = {"f32": f32, "f32r": dt.float32r, "bf16": bf16}[PROJ_DTYPE]

    nc = bacc.Bacc("TRN2", debug=False)

    xT_d = nc.dram_tensor("xT", [128, CC, T * BL], f32, kind="ExternalInput").ap()
    wihT_d = nc.dram_tensor("wihT", [128, CC, N], f32, kind="ExternalInput").ap()
    whhT_d = nc.dram_tensor(
        "whhT", [128, KC, N], rec_w_dt, kind="ExternalInput"
    ).ap()
    out_d = nc.dram_tensor("out", [128, KC, T * BL], f32, kind="ExternalOutput").ap()

    with tile.TileContext(nc) as tc:
        with (
            tc.tile_pool(name="consts", bufs=1) as consts,
            tc.tile_pool(name="xp", bufs=1) as xp_pool,
            tc.tile_pool(name="hstage", bufs=2) as h_pool,
            tc.tile_pool(name="a", bufs=3) as a_pool,
            tc.tile_pool(name="psum_p", bufs=2, space="PSUM") as psum_p,
            tc.tile_pool(name="psum_r", bufs=6, space="PSUM") as psum_r,
        ):
            # ---- load inputs ----
            xT = consts.tile([128, CC, T * BL], f32)
            nc.sync.dma_start(xT[:], xT_d[:])
            wihT = consts.tile([128, CC, N], f32)
            nc.sync.dma_start(wihT[:], wihT_d[:])
            whhT = consts.tile([128, KC, N], rec_w_dt)
            nc.sync.dma_start(whhT[:], whhT_d[:])

            # ---- input projection: xpT[m][:, t*BL+b] for all t ----
            xpT = xp_pool.tile([128, KC, T * BL], f32)
            NSL = 512  # moving columns per matmul
            nslices = (T * BL) // NSL
            for m in range(KC):
                for s in range(nslices):
                    ps = psum_p.tile([128, NSL], f32)
                    for k in range(CC):
                        nc.tensor.matmul(
                            ps[:],
                            wihT[:, k, m * 128 : (m + 1) * 128].bitcast(proj_mm_dt),
                            xT[:, k, s * NSL : (s + 1) * NSL].bitcast(proj_mm_dt),
                            start=(k == 0),
                            stop=(k == CC - 1),
                        )
                    nc.vector.tensor_copy(
                        xpT[:, m, s * NSL : (s + 1) * NSL], ps[:]
                    )

            # ---- recurrence ----
            a_zero = consts.tile([128, KC, BL], rec_w_dt)
            nc.any.memset(a_zero[:], 0.0)
            a_prev = a_zero

            for blk in range(T // TBLK):
                hT = h_pool.tile([128, KC, TBLK * BL], f32)
                for tt in range(TBLK):
                    t = blk * TBLK + tt
                    ps_m = [
                        psum_r.tile([128, BL], f32, tag="psr", name=f"psr{_m}")
                        for _m in range(KC)
                    ]
                    a_next = a_pool.tile([128, KC, BL], rec_w_dt)
                    # k-major: psum groups interleave so the PE never waits
                    # on the latest tanh chunk.
                    for k in range(KC):
                        for m in range(KC):
                            nc.tensor.matmul(
                                ps_m[m][:],
                                whhT[:, k, m * 128 : (m + 1) * 128].bitcast(rec_mm_dt),
                                a_prev[:, k, :].bitcast(rec_mm_dt),
                                start=(k == 0),
                                stop=(k == KC - 1),
                                skip_group_check=True,
                            )
                    for m in range(KC):
                        hsl = hT[:, m, tt * BL : (tt + 1) * BL]
                        nc.vector.tensor_add(
                            hsl, ps_m[m][:], xpT[:, m, t * BL : (t + 1) * BL]
                        )
                        nc.scalar.activation(
                            a_next[:, m, :],
                            hsl,
                            mybir.ActivationFunctionType.Tanh,
                        )
                    a_prev = a_next
                nc.sync.dma_start(
                    out_d[:, :, blk * TBLK * BL : (blk + 1) * TBLK * BL], hT[:]
                )

    nc.compile()
    return nc


def _get_nc():
    key = (REC_DTYPE, PROJ_DTYPE)
    if key not in _CACHE:
        _CACHE[key] = _build()
    return _CACHE[key]


def _prep_core_inputs(x, W_ih, W_hh):
    """Host-side shard + transpose into the kernel's DRAM layouts."""
    wihT = (
        np.ascontiguousarray(W_ih.T.astype(np.float32))
        .reshape(CC, 128, N)
        .transpose(1, 0, 2)
    )
    wihT = np.ascontiguousarray(wihT)
    w_dt = np.float32 if REC_DTYPE in ("f32", "f32r") else ml_dtypes.bfloat16
    whhT = (
        np.ascontiguousarray(W_hh.T)
        .reshape(KC, 128, N)
        .transpose(1, 0, 2)
    )
    whhT = np.ascontiguousarray(whhT).astype(w_dt)

    in_maps = []
    for c in range(NCORES):
        xc = x[c * BL : (c + 1) * BL]  # [BL, T, NIN]
        xTc = np.ascontiguousarray(
            xc.transpose(2, 1, 0).reshape(CC, 128, T * BL).transpose(1, 0, 2)
        ).astype(np.float32)
        in_maps.append({"xT": xTc, "wihT": wihT, "whhT": whhT})
    return in_maps


def kernel(x, W_ih, W_hh):
    from concourse.bass_utils import run_bass_kernel_spmd

    x = np.asarray(x, dtype=np.float32)
    W_ih = np.asarray(W_ih, dtype=np.float32)
    W_hh = np.asarray(W_hh, dtype=np.float32)

    nc = _get_nc()
    in_maps = _prep_core_inputs(x, W_ih, W_hh)
    res = run_bass_kernel_spmd(nc, in_maps, core_ids=list(range(NCORES)))

    out = np.empty((B, T, N), dtype=np.float32)
    for c in range(NCORES):
        o = res.results[c]["out"]  # [128, KC, T*BL]
        o = o.reshape(128, KC, T, BL).transpose(3, 2, 1, 0).reshape(BL, T, N)
        out[c * BL : (c + 1) * BL] = o
    return out


if __name__ == "__main__":
    xs = np.random.randn(B, T, NIN).astype(np.float32)
    wi = np.random.randn(N, NIN).astype(np.float32) / np.sqrt(NIN)
    wh = np.random.randn(N, N).astype(np.float32) / np.sqrt(N)
    r = kernel(xs, wi, wh)
    print("kernel ran, out shape", r.shape, "mean", float(np.abs(r).mean()))


# revision 11
# speedup vs baseline: 10253.7063x; 10253.7063x over previous
"""Trainium2 Bass kernel for a basic RNN:
    h_t = W_hh @ tanh(h_{t-1}) + W_ih @ x_t   (pre-activation hidden stored)
    x: [B=64, T=512, NIN=256] fp32, W_ih: [512, 256], W_hh: [512, 512]
    out: [B, T, N=512] fp32

Strategy
--------
Data-parallel over batch: B=64 -> 8 cores x BL=8 sequences each.
Per core everything is kept in a hidden-major ("transposed") layout
[hidden (partition), batch (free)] so the sequential recurrence needs no
per-step transposes:

  - host pre-transposes x to xT[c, t*BL+b] and the weights to W.T
  - PE computes xp.T = W_ih.T^T @ x.T for all steps (one big matmul)
  - recurrence: for each t, for each output chunk m (4x128):
        psum[m] = sum_k W_hh.T[k-chunk, m-chunk]^T @ tanh(h_{t-1}).T[k-chunk]
    with W_hh tiles as the stationary operand (bf16 -> fast weight load)
    and the tiny [128, BL] activation as the moving operand.
    DVE adds xp_t, ACT applies tanh (emitting bf16 for the next step).
  - h_t (fp32, hidden-major) is staged in SBUF and DMA'd out in blocks;
    the host transposes back to [B, T, N].

The W_hh stream through the PE array (128 elem/cycle) is the intrinsic
floor: ~853ns/step * 512 steps.
"""

import os
import numpy as np
import ml_dtypes

B, T, NIN, N = 64, 512, 256, 512
NCORES = 8
BL = B // NCORES  # 8 sequences per core
KC = N // 128  # 4 hidden chunks
CC = NIN // 128  # 2 input-feature chunks
TBLK = 64  # steps staged in SBUF between output DMAs

# "bf16" (fast) or "f32" (exact, ~4x slower recurrence) or "f32r"
REC_DTYPE = os.environ.get("RNN_REC_DTYPE", "bf16")
PROJ_DTYPE = os.environ.get("RNN_PROJ_DTYPE", "bf16")
KVER = os.environ.get("RNN_KVER", "v3")

_CACHE = {}


def _build(rec_dtype, proj_dtype, repeat=1, mini=False):
    """Build + compile the per-core Bass program.

    repeat: run the recurrence phase `repeat` times (for differential
        wall-clock timing; outputs are overwritten identically).
    mini: only 16 recurrence steps (structurally identical kernel for
        calibrating dispatch + transfer + setup overhead).
    """
    import concourse.bacc as bacc
    import concourse.mybir as mybir
    from concourse import tile

    dt = mybir.dt
    f32 = dt.float32

    rec_mm_dt = {"bf16": dt.bfloat16, "f32": f32, "f32r": dt.float32r}[rec_dtype]
    proj_mm_dt = {"f32": f32, "f32r": dt.float32r, "bf16": dt.bfloat16}[proj_dtype]

    nc = bacc.Bacc("TRN2", debug=False)

    xT_d = nc.dram_tensor(
        "xT", [128, CC, T * BL], proj_mm_dt, kind="ExternalInput"
    ).ap()
    wihT_d = nc.dram_tensor("wihT", [128, CC, N], proj_mm_dt, kind="ExternalInput").ap()
    whhT_d = nc.dram_tensor("whhT", [128, KC, N], rec_mm_dt, kind="ExternalInput").ap()
    out_d = nc.dram_tensor("out", [128, KC, T * BL], f32, kind="ExternalOutput").ap()

    n_blks = 1 if mini else T // TBLK
    tblk = 16 if mini else TBLK
    nstream = 2 if KVER == "v3" else 1
    sb = BL // nstream  # batch columns per stream

    with tile.TileContext(nc) as tc:
        with (
            tc.tile_pool(name="consts", bufs=1) as consts,
            tc.tile_pool(name="hstage", bufs=2) as h_pool,
            tc.tile_pool(name="a", bufs=4) as a_pool,
            tc.tile_pool(name="psum_r", bufs=8, space="PSUM") as psum_r,
        ):
            # ---- load inputs ----
            xT = consts.tile([128, CC, T * BL], proj_mm_dt)
            nc.sync.dma_start(xT[:], xT_d[:])
            wihT = consts.tile([128, CC, N], proj_mm_dt)
            nc.sync.dma_start(wihT[:], wihT_d[:])
            whhT = consts.tile([128, KC, N], rec_mm_dt)
            nc.sync.dma_start(whhT[:], whhT_d[:])

            a_zero = consts.tile([128, KC, BL], rec_mm_dt)
            nc.any.memset(a_zero[:], 0.0)

            # Per step and stream: 8 projection MMs (independent of the
            # recurrence -> fill the tanh-chain gap), 16 recurrence MMs,
            # then ONE tanh (ACT reads PSUM) and ONE fp32 copy (DVE reads
            # PSUM) -- ACT is not behind DVE on the critical path.
            for rep in range(repeat):
                a_prev = [a_zero[:, :, s * sb : (s + 1) * sb] for s in range(nstream)]
                for blk in range(n_blks):
                    hT = h_pool.tile([128, KC, tblk * BL], f32, tag="hT", name="hT")
                    for tt in range(tblk):
                        t = blk * tblk + tt
                        for s in range(nstream):
                            c0 = t * BL + s * sb  # column base in xT
                            ps = psum_r.tile(
                                [128, KC, sb], f32, tag="psr", name="psr"
                            )
                            for k2 in range(CC):
                                for m in range(KC):
                                    nc.tensor.matmul(
                                        ps[:, m, :],
                                        wihT[:, k2, m * 128 : (m + 1) * 128],
                                        xT[:, k2, c0 : c0 + sb],
                                        start=(k2 == 0 and m == 0),
                                        stop=False,
                                        skip_group_check=True,
                                    )
                            for k in range(KC):
                                for m in range(KC):
                                    nc.tensor.matmul(
                                        ps[:, m, :],
                                        whhT[:, k, m * 128 : (m + 1) * 128],
                                        a_prev[s][:, k, :],
                                        start=False,
                                        stop=(k == KC - 1),
                                        skip_group_check=True,
                                    )
                            a_next = a_pool.tile(
                                [128, KC, sb], rec_mm_dt, tag=f"aT{s}", name="aT"
                            )
                            nc.scalar.activation(
                                a_next[:], ps[:], mybir.ActivationFunctionType.Tanh
                            )
                            nc.vector.tensor_copy(
                                hT[:, :, tt * BL + s * sb : tt * BL + (s + 1) * sb],
                                ps[:],
                            )
                            a_prev[s] = a_next[:]
                    nc.sync.dma_start(
                        out_d[:, :, blk * tblk * BL : (blk + 1) * tblk * BL], hT[:]
                    )

    nc.compile()
    return nc


class Runner:
    """Persistent jitted SPMD executor over the 8 NeuronCores.

    Replicates bass2jax.run_bass_via_pjrt's lowering but keeps the jitted
    callable and device buffers alive so repeated calls measure execution
    (not retrace/transfer).
    """

    def __init__(self, nc):
        import jax
        import jax.numpy as jnp
        from jax.experimental.shard_map import shard_map
        from jax.sharding import Mesh, NamedSharding, PartitionSpec
        import concourse.mybir as mybir
        from concourse import bass2jax

        bass2jax.install_neuronx_cc_hook()
        self.jax = jax
        self.nc = nc

        partition_name = (
            nc.partition_id_tensor.name if nc.partition_id_tensor else None
        )
        in_names, out_names, out_avals = [], [], []
        for alloc in nc.m.functions[0].allocations:
            if not isinstance(alloc, mybir.MemoryLocationSet):
                continue
            name = alloc.memorylocations[0].name
            if alloc.kind == "ExternalInput":
                if name != partition_name:
                    in_names.append(name)
            elif alloc.kind == "ExternalOutput":
                out_names.append(name)
                out_avals.append(
                    jax.core.ShapedArray(
                        tuple(alloc.tensor_shape), mybir.dt.np(alloc.dtype)
                    )
                )
        self.in_names = list(in_names)
        self.out_names = list(out_names)
        self.out_avals = out_avals
        n_params = len(in_names)
        all_in_names = in_names + out_names
        if partition_name is not None:
            all_in_names = all_in_names + [partition_name]

        def _body(*args):
            operands = list(args)
            if partition_name is not None:
                operands.append(bass2jax.partition_id_tensor())
            outs = bass2jax._bass_exec_p.bind(
                *operands,
                out_avals=tuple(out_avals),
                in_names=tuple(all_in_names),
                out_names=tuple(self.out_names),
                lowering_input_output_aliases=(),
                sim_require_finite=True,
                sim_require_nnan=True,
                nc=nc,
            )
            return tuple(outs)

        devices = jax.devices()[:NCORES]
        self.mesh = Mesh(np.asarray(devices), ("core",))
        self.sharding = NamedSharding(self.mesh, PartitionSpec("core"))
        n_outs = len(out_names)
        self.fn = jax.jit(
            shard_map(
                _body,
                mesh=self.mesh,
                in_specs=(PartitionSpec("core"),) * (n_params + n_outs),
                out_specs=(PartitionSpec("core"),) * n_outs,
                check_rep=False,
            ),
            keep_unused=True,
        )
        # reusable on-device zero output buffers (not donated)
        self.zero_outs = [
            jax.device_put(
                np.zeros((NCORES * a.shape[0], *a.shape[1:]), a.dtype), self.sharding
            )
            for a in out_avals
        ]

    def put(self, in_maps):
        concat = [
            np.concatenate([np.asarray(m[name]) for m in in_maps], axis=0)
            for name in self.in_names
        ]
        return [self.jax.device_put(a, self.sharding) for a in concat]

    def run(self, dev_in):
        outs = self.fn(*dev_in, *self.zero_outs)
        self.jax.block_until_ready(outs)
        return outs

    def run_np(self, dev_in):
        outs = self.run(dev_in)
        res = []
        for c in range(NCORES):
            res.append(
                {
                    name: np.asarray(outs[i]).reshape(
                        NCORES, *self.out_avals[i].shape
                    )[c]
                    for i, name in enumerate(self.out_names)
                }
            )
        return res


def get_runner(rec_dtype=None, proj_dtype=None, repeat=1, mini=False):
    key = (rec_dtype or REC_DTYPE, proj_dtype or PROJ_DTYPE, repeat, mini)
    if key not in _CACHE:
        nc = _build(*key)
        _CACHE[key] = Runner(nc)
    return _CACHE[key]


def prep_inputs(x, W_ih, W_hh, rec_dtype=None, proj_dtype=None):
    """Host-side shard + transpose into the kernel's DRAM layouts."""
    rec_dtype = rec_dtype or REC_DTYPE
    proj_dtype = proj_dtype or PROJ_DTYPE
    p_np = ml_dtypes.bfloat16 if proj_dtype == "bf16" else np.float32
    w_np = ml_dtypes.bfloat16 if rec_dtype == "bf16" else np.float32
    wihT = np.ascontiguousarray(
        np.ascontiguousarray(W_ih.T.astype(np.float32))
        .reshape(CC, 128, N)
        .transpose(1, 0, 2)
    ).astype(p_np)
    whhT = np.ascontiguousarray(
        np.ascontiguousarray(W_hh.T).reshape(KC, 128, N).transpose(1, 0, 2)
    ).astype(w_np)

    in_maps = []
    for c in range(NCORES):
        xc = x[c * BL : (c + 1) * BL]  # [BL, T, NIN]
        xTc = np.ascontiguousarray(
            xc.transpose(2, 1, 0).reshape(CC, 128, T * BL).transpose(1, 0, 2)
        ).astype(p_np)
        in_maps.append({"xT": xTc, "wihT": wihT, "whhT": whhT})
    return in_maps


def gather_output(res):
    out = np.empty((B, T, N), dtype=np.float32)
    for c in range(NCORES):
        o = res[c]["out"]  # [128, KC, T*BL]
        o = o.reshape(128, KC, T, BL).transpose(3, 2, 1, 0).reshape(BL, T, N)
        out[c * BL : (c + 1) * BL] = o
    return out


def kernel(x, W_ih, W_hh):
    x = np.asarray(x, dtype=np.float32)
    W_ih = np.asarray(W_ih, dtype=np.float32)
    W_hh = np.asarray(W_hh, dtype=np.float32)

    runner = get_runner()
    dev_in = runner.put(prep_inputs(x, W_ih, W_hh))
    res = runner.run_np(dev_in)
    return gather_output(res)


if __name__ == "__main__":
    xs = np.random.randn(B, T, NIN).astype(np.float32)
    wi = (np.random.randn(N, NIN) / np.sqrt(NIN)).astype(np.float32)
    wh = (np.random.randn(N, N) / np.sqrt(N)).astype(np.float32)
    r = kernel(xs, wi, wh)
    print("kernel ran, out shape", r.shape, "mean", float(np.abs(r).mean()))
